# revision 14
# baseline (speedup 1.0000x reference)
"""Multi-head attention (B=8, N=1024, D=1024, H=16) on 8 TRN2 NeuronCores.

Sharding: data-parallel over batch - core i computes batch item i end-to-end.
No collectives.

v2 pipeline (all matmul stationary operands bf16 -> FWL weight loads):
  A)  DMA x first; cast x->bf16 (DVE); PE transpose -> xT bf16
      weights staged fp32 then cast to bf16 on GpSimd (idle engine)
  B)  qkproj pair0 (lhsT=wq bf16, rhs=xT bf16) -> qT,kT bf16
  C)  128 attention steps (pair, cpass, j), software-pipelined with
      score emitted one step ahead of AV so ACT (exp) never stalls:
        S^T[k,q] pair-packed K=64 row groups; exp via ACT -> es bf16
        O'^T[65,q] += V'_h^T es  (V' carries a ones column -> row 64 = sums)
      V projection interleaved into pair0 cpass0 (V[j] before AV j).
      Per-pair normalization: sums rows -> DRAM -> reciprocal [4,512] ->
      bf16 -> DRAM -> partition-broadcast -> Onorm *= rec (pair 7 split
      into two [2,512] chains to shorten the tail).
  D)  out = Onorm^T @ w_proj + b  (lhsT=Onorm bf16, rhs=wp bf16)

softmax max-subtraction skipped: scores ~N(0, 0.33^2), safe for exp in fp32.
"""

import os
import sys
import types

sys.path.insert(0, "/opt/trn_rl_repo")

# The agent image's antenv lacks axon_hooks; register the NTFF profile hook
# shim so run_bass_kernel_spmd(trace=True) can capture exec_time_ns.
if "antenv.axon_hooks" not in sys.modules:
    _hooks = types.ModuleType("antenv.axon_hooks")
    _hook_store = [None]
    _hooks.set_axon_ntff_profile_hook = lambda h: _hook_store.__setitem__(0, h)
    _hooks.get_axon_ntff_profile_hook = lambda: _hook_store[0]
    sys.modules["antenv.axon_hooks"] = _hooks
    try:
        from trn_agent_boot.trn_boot import _ntff_profile_via_ctypes

        _hooks.set_axon_ntff_profile_hook(
            _ntff_profile_via_ctypes("/opt/axon/libaxon_pjrt.so")
        )
    except Exception:
        pass

import numpy as np
import concourse.bass as bass
import concourse.bacc as bacc
import concourse.mybir as mybir
import concourse.tile as tile
from concourse import masks
from concourse.bass_utils import run_bass_kernel_spmd

F32 = mybir.dt.float32
F32R = mybir.dt.float32r
BF16 = mybir.dt.bfloat16
EXP = mybir.ActivationFunctionType.Exp

B = 8
N = 1024  # sequence length
D = 1024  # embed dim
H = 16  # heads
HD = 64  # head dim
SCALE = HD**-0.5  # 0.125
NT = N // 128  # 8 seq tiles
DT = D // 128  # 8 embed tiles
NC2 = N // 512  # 2 free-dim chunks of 512
NPAIR = H // 2  # 8 head pairs

LAST_EXEC_NS = [None]


def build():
    nc = bacc.Bacc(None, target_bir_lowering=False)
    x = nc.declare_dram_parameter("x", [N, D], F32, isOutput=False)
    w_qkv = nc.declare_dram_parameter("w_qkv", [D, 3 * D], F32, isOutput=False)
    w_proj = nc.declare_dram_parameter("w_proj", [D, D], F32, isOutput=False)
    b_proj = nc.declare_dram_parameter("b_proj", [D], F32, isOutput=False)
    out = nc.declare_dram_parameter("out", [N, D], F32, isOutput=True)

    with tile.TileContext(nc) as tc:
        with (
            tc.tile_pool(name="const", bufs=1) as cpool,
            tc.tile_pool(name="xT", bufs=DT) as xTpool,
            tc.tile_pool(name="V", bufs=NT) as Vpool,
            tc.tile_pool(name="qkT", bufs=4) as qkTpool,
            tc.tile_pool(name="Onorm", bufs=DT) as Opool,
            tc.tile_pool(name="es", bufs=5) as espool,
            tc.tile_pool(name="wqk", bufs=4) as wqkpool,
            tc.tile_pool(name="wv", bufs=DT) as wvpool,
            tc.tile_pool(name="wp", bufs=DT) as wppool,
            tc.tile_pool(name="wst", bufs=4) as wstpool,
            tc.tile_pool(name="sums", bufs=7) as sumspool,
            tc.tile_pool(name="recb", bufs=3) as rbpool,
            tc.tile_pool(name="drs", bufs=1, space="DRAM") as drpool,
            tc.tile_pool(name="pq", bufs=1, space="PSUM") as pqpool,
        ):
            ident = cpool.tile([128, 128], F32, tag="ident")
            masks.make_identity(nc, ident[:])
            ident_r = cpool.tile([128, 128], F32R, tag="ident_r")
            nc.vector.tensor_copy(ident_r[:], ident[:])
            onescf = cpool.tile([128, H], F32, tag="onescf")
            nc.vector.memset(onescf[:], 1.0)
            b_bc = cpool.tile([128, D], F32, tag="b_bc")

            xT = [
                xTpool.tile([128, N], BF16, tag="xT", name=f"xT{j}")
                for j in range(DT)
            ]
            V = [
                Vpool.tile([128, H * (HD + 1)], BF16, tag="V", name=f"V{i}")
                for i in range(NT)
            ]
            Onorm = [
                Opool.tile([128, N], BF16, tag="On", name=f"On{i}")
                for i in range(DT)
            ]
            wv = [
                wvpool.tile([128, D], BF16, tag="wv", name=f"wv{j}")
                for j in range(DT)
            ]
            wp = [
                wppool.tile([128, D], BF16, tag="wp", name=f"wp{ct}")
                for ct in range(DT)
            ]
            sums_dram = drpool.tile([4 * NPAIR, 512], F32, tag="sumsd")
            rec_dram = drpool.tile([4 * NPAIR, 512], BF16, tag="recd")

            def stage_qk(pr):
                # enqueue fp32 stage DMAs for wq/wk column slices
                stgs = []
                for et in (pr, NPAIR + pr):
                    stg = wstpool.tile([128, D], F32, tag="wst", name=f"ws{et}")
                    src_w = w_qkv[:, et * 128 : (et + 1) * 128].rearrange(
                        "(j p) e -> p j e", p=128
                    )
                    nc.sync.dma_start(
                        stg[:].rearrange("p (j e) -> p j e", e=128), src_w
                    )
                    stgs.append(stg)
                return stgs

            def make_qk_emitter(pr, stgs=None):
                # cast staged weights to bf16 (DVE); emit(n) issues n matmuls
                # (32 total = 2 etiles x 2 chunks x 8 j)
                if stgs is None:
                    stgs = stage_qk(pr)
                wqs = []
                for k, et in enumerate((pr, NPAIR + pr)):
                    wq = wqkpool.tile([128, D], BF16, tag="wqk", name=f"wq{et}")
                    nc.vector.tensor_copy(wq[:], stgs[k][:])
                    wqs.append(wq)
                qts = [
                    qkTpool.tile([128, N], BF16, tag="qkT", name=f"qt{pr}"),
                    qkTpool.tile([128, N], BF16, tag="qkT", name=f"kt{pr}"),
                ]
                state = {"idx": 0, "pq": None}

                def emit(n):
                    for _ in range(n):
                        idx = state["idx"]
                        if idx >= 32:
                            return
                        g, j = idx // DT, idx % DT
                        ei, c = ((0, 0), (1, 0), (1, 1), (0, 1))[g]
                        if j == 0:
                            state["pq"] = pqpool.tile(
                                [128, 512], F32, tag="pq", name="pq"
                            )
                        nc.tensor.matmul(
                            state["pq"][:],
                            wqs[ei][:, j * 128 : (j + 1) * 128],
                            xT[j][:, c * 512 : (c + 1) * 512],
                            start=(j == 0),
                            stop=(j == DT - 1),
                        )
                        if j == DT - 1:
                            nc.vector.tensor_copy(
                                qts[ei][:, c * 512 : (c + 1) * 512],
                                state["pq"][:],
                            )
                        state["idx"] = idx + 1

                return emit, qts

            # ---- phase A: load x, cast bf16, transpose; stage weights ----
            with (
                tc.tile_pool(name="xin", bufs=6) as xpool,
                tc.tile_pool(name="tp", bufs=2, space="PSUM") as tppool,
            ):
                # DMA order: all of x first, then wq pair0, then wv, b_bc
                xts = []
                for i in range(4):
                    xt = xpool.tile([128, D], F32R, tag="x", name=f"x{i}")
                    nc.sync.dma_start(
                        xt[:], x[i * 128 : (i + 1) * 128, :].bitcast(F32R)
                    )
                    xts.append(xt)
                stg0 = stage_qk(0)
                for i in range(4, NT):
                    xt = xpool.tile([128, D], F32R, tag="x", name=f"x{i}")
                    nc.sync.dma_start(
                        xt[:], x[i * 128 : (i + 1) * 128, :].bitcast(F32R)
                    )
                    xts.append(xt)
                wvstg = []
                for j in range(DT):
                    stg = wstpool.tile([128, D], F32, tag="wst", name=f"wvs{j}")
                    nc.sync.dma_start(
                        stg[:], w_qkv[j * 128 : (j + 1) * 128, 2 * D : 3 * D]
                    )
                    wvstg.append(stg)
                nc.sync.dma_start(
                    b_bc[:].rearrange("p (a f) -> p a f", a=1),
                    b_proj[:].rearrange("(a n) -> a n", a=1).partition_broadcast(
                        128
                    ),
                )

                def transpose_tile(i):
                    for j in range(DT):
                        tp = tppool.tile([128, 128], F32R, tag="tp")
                        nc.tensor.transpose(
                            tp[:], xts[i][:, j * 128 : (j + 1) * 128], ident_r[:]
                        )
                        nc.vector.tensor_copy(
                            xT[j][:, i * 128 : (i + 1) * 128],
                            tp[:].bitcast(F32),
                        )

                for i in range(4):
                    transpose_tile(i)
                # wq0 casts land on DVE right after x0-3 copies
                emit0, qts0 = make_qk_emitter(0, stg0)
                for i in range(4, NT):
                    transpose_tile(i)

                # qk projection for pair 0: first q-c0 + k-c0 groups, so
                # scores can start; the rest drips into the step loop
                emit0(18)

                for j in range(DT):
                    nc.vector.tensor_copy(wv[j][:], wvstg[j][:])

                # ones columns of V' (col 64 of each 65-wide head group)
                for i in range(NT):
                    ones_view = V[i][:].rearrange("p (h e) -> p h e", e=HD + 1)
                    nc.vector.tensor_copy(
                        ones_view[:, :, HD : HD + 1],
                        onescf[:].rearrange("p (h e) -> p h e", e=1),
                    )

            # ---- phase C: attention steps, software pipelined ------------
            with (
                tc.tile_pool(name="s", bufs=2, space="PSUM") as spool,
                tc.tile_pool(name="o", bufs=3, space="PSUM") as opool,
            ):
                steps = [
                    (p, c, j)
                    for p in range(NPAIR)
                    for c in range(NC2)
                    for j in range(NT)
                ]
                qts_of = {0: qts0}
                emitters = {}

                def emit_score(s):
                    p, c, j = steps[s]
                    qt, kt = qts_of[p]
                    st = spool.tile([128, N], F32, tag="s", name="st")
                    cs = slice(c * 512, (c + 1) * 512)
                    for parity in range(2):
                        p0 = 64 * parity
                        nc.tensor.matmul(
                            st[:, 512 * parity : 512 * parity + 512],
                            kt[p0 : p0 + 64, j * 128 : (j + 1) * 128],
                            qt[p0 : p0 + 64, cs],
                            start=True,
                            stop=True,
                        )
                    return st

                def emit_vproj(i):
                    dst = V[i][:].rearrange("p (h e) -> p h e", e=HD + 1)
                    for cc in range(NC2):
                        pv = pqpool.tile([128, 512], F32, tag="pq", name="pv")
                        for jj in range(DT):
                            nc.tensor.matmul(
                                pv[:],
                                xT[jj][:, i * 128 : (i + 1) * 128],
                                wv[jj][:, cc * 512 : (cc + 1) * 512],
                                start=(jj == 0),
                                stop=(jj == DT - 1),
                            )
                        nc.vector.tensor_copy(
                            dst[:, 8 * cc : 8 * cc + 8, 0:HD],
                            pv[:].rearrange("p (h e) -> p h e", e=HD),
                        )

                def emit_chain(p, sld, base, nrows, eng):
                    rec = sumspool.tile([nrows, 512], F32, tag="rec", name="rec")
                    nc.vector.reciprocal(rec[:], sld[:])
                    r16 = sumspool.tile([nrows, 512], BF16, tag="r16", name="r16")
                    eng.tensor_copy(r16[:], rec[:])
                    nc.sync.dma_start(rec_dram[base : base + nrows, :], r16[:])
                    for r in range(nrows):
                        row = base + r
                        cc, parity = (row % 4) // 2, row % 2
                        p0 = 64 * parity
                        rb = rbpool.tile([128, 512], BF16, tag="rb")
                        half = rb[p0 : p0 + 64, :]
                        nc.sync.dma_start(
                            half.rearrange("p (a f) -> p a f", a=1),
                            rec_dram[row : row + 1, :].partition_broadcast(64),
                        )
                        osl = Onorm[p][p0 : p0 + 64, cc * 512 : (cc + 1) * 512]
                        eng.tensor_mul(osl, osl, half)

                pending = {}
                st_cur = emit_score(0)
                po = None
                sums_sb = None
                rec16 = None
                for s, (pair, c, j) in enumerate(steps):
                    if c == 0 and j == 0:
                        # pair start: prefetch next pair's wq; wp at pairs 2-5
                        if pair + 1 < NPAIR:
                            emitters[pair + 1] = make_qk_emitter(pair + 1)
                            qts_of[pair + 1] = emitters[pair + 1][1]
                        if 2 <= pair <= 5:
                            for ct in (2 * pair - 4, 2 * pair - 3):
                                stg = wstpool.tile(
                                    [128, D], F32, tag="wst", name=f"wps{ct}"
                                )
                                nc.sync.dma_start(
                                    stg[:],
                                    w_proj[ct * 128 : (ct + 1) * 128, :],
                                )
                                nc.gpsimd.tensor_copy(wp[ct][:], stg[:])
                    # exp of this step's scores (ACT paces the pipeline)
                    es = espool.tile([128, N], BF16, tag="es")
                    nc.scalar.activation(es[:], st_cur[:], EXP, scale=SCALE)

                    # lead: next step's score matmuls ahead of this AV
                    if s + 1 < len(steps):
                        st_cur = emit_score(s + 1)

                    if j == 0:
                        po = [
                            opool.tile([128, 512], F32, tag="o", name=f"po{k}")
                            for k in range(2)
                        ]
                    # V projection interleaved into pair0 cpass0
                    if pair == 0 and c == 0:
                        emit_vproj(j)

                    for parity in range(2):
                        h = 2 * pair + parity
                        nc.tensor.matmul(
                            po[parity][0 : HD + 1, :],
                            V[j][:, h * (HD + 1) : (h + 1) * (HD + 1)],
                            es[:, 512 * parity : 512 * parity + 512],
                            start=(j == 0),
                            stop=(j == NT - 1),
                        )
                    if pair == 0:
                        emit0(3)
                    if pair + 1 in emitters and (c * NT + j) >= 3:
                        emitters[pair + 1][0](3)

                    if j == NT - 1:
                        # drain po: O^T rows + sums row; per-cpass norm chain
                        cs = slice(c * 512, (c + 1) * 512)
                        for parity in range(2):
                            p0 = 64 * parity
                            nc.vector.tensor_copy(
                                Onorm[pair][p0 : p0 + 64, cs],
                                po[parity][0:HD, :],
                            )
                            srow = sumspool.tile([1, 512], F32, tag="srow")
                            nc.vector.tensor_copy(
                                srow[:], po[parity][HD : HD + 1, :]
                            )
                            row = 4 * pair + 2 * c + parity
                            nc.sync.dma_start(
                                sums_dram[row : row + 1, :], srow[:]
                            )
                        # enqueue the sums row-gather; the reciprocal +
                        # muls are deferred so they never block the pipe
                        if pair < NPAIR - 1:
                            if c == 1:
                                sld = sumspool.tile(
                                    [4, 512], F32, tag="sld", name="sld"
                                )
                                nc.sync.dma_start(
                                    sld[:], sums_dram[4 * pair : 4 * pair + 4, :]
                                )
                                pending[pair] = sld
                        else:
                            sld = sumspool.tile(
                                [2, 512], F32, tag="sld", name="sld7"
                            )
                            base = 4 * pair + 2 * c
                            nc.sync.dma_start(
                                sld[:], sums_dram[base : base + 2, :]
                            )
                            if c == 0:
                                pending[(NPAIR - 1, 0)] = sld
                            else:
                                # tail chain: all on DVE, minimal latency
                                emit_chain(pair, sld, base, 2, nc.vector)
                    if c == 0 and j == 2 and pair >= 1 and (pair - 1) in pending:
                        emit_chain(
                            pair - 1, pending.pop(pair - 1), 4 * (pair - 1),
                            4, nc.gpsimd,
                        )
                    if (
                        c == 1
                        and j == 2
                        and pair == NPAIR - 1
                        and (NPAIR - 1, 0) in pending
                    ):
                        emit_chain(
                            pair, pending.pop((NPAIR - 1, 0)), 4 * pair, 2,
                            nc.gpsimd,
                        )

            # ---- phase D: output projection + bias -----------------------
            with (
                tc.tile_pool(name="osb", bufs=2) as osbpool,
                tc.tile_pool(name="dp", bufs=4, space="PSUM") as dppool,
            ):
                for i in range(NT):
                    pd = [
                        dppool.tile([128, 512], F32, tag="dp", name=f"dp{ec}")
                        for ec in range(NC2)
                    ]
                    for ct in range(DT):
                        for ec in range(NC2):
                            nc.tensor.matmul(
                                pd[ec][:],
                                Onorm[ct][:, i * 128 : (i + 1) * 128],
                                wp[ct][:, ec * 512 : (ec + 1) * 512],
                                start=(ct == 0),
                                stop=(ct == DT - 1),
                            )
                    ot = osbpool.tile([128, D], F32, tag="osb")
                    for ec in range(NC2):
                        nc.vector.tensor_add(
                            ot[:, ec * 512 : (ec + 1) * 512],
                            pd[ec][:],
                            b_bc[:, ec * 512 : (ec + 1) * 512],
                        )
                    nc.sync.dma_start(out[i * 128 : (i + 1) * 128, :], ot[:])

    nc.compile()
    return nc


_NC = [None]


def _get_nc():
    if _NC[0] is None:
        _NC[0] = build()
    return _NC[0]


def kernel(x, w_qkv, w_proj, b_proj):
    x = np.asarray(x, dtype=np.float32)
    w_qkv = np.asarray(w_qkv, dtype=np.float32)
    w_proj = np.asarray(w_proj, dtype=np.float32)
    b_proj = np.asarray(b_proj, dtype=np.float32)
    assert x.shape == (B, N, D)

    nc = _get_nc()
    in_maps = [
        {"x": x[i], "w_qkv": w_qkv, "w_proj": w_proj, "b_proj": b_proj}
        for i in range(B)
    ]
    trace = os.environ.get("KERNEL_TRACE") == "1"
    res = run_bass_kernel_spmd(
        nc, in_maps, core_ids=list(range(B)), trace=trace
    )
    LAST_EXEC_NS[0] = res.exec_time_ns
    return np.stack([res.results[i]["out"] for i in range(B)], axis=0)


# revision 15
# speedup vs baseline: 1.1560x; 1.1560x over previous
"""Multi-head attention (B=8, N=1024, D=1024, H=16) on 8 TRN2 NeuronCores.

Sharding: data-parallel over batch - core i computes batch item i end-to-end.
No collectives.

v2 pipeline (all matmul stationary operands bf16 -> FWL weight loads):
  A)  DMA x first; cast x->bf16 (DVE); PE transpose -> xT bf16
      weights staged fp32 then cast to bf16 on GpSimd (idle engine)
  B)  qkproj pair0 (lhsT=wq bf16, rhs=xT bf16) -> qT,kT bf16
  C)  128 attention steps (pair, cpass, j), software-pipelined with
      score emitted one step ahead of AV so ACT (exp) never stalls:
        S^T[k,q] pair-packed K=64 row groups; exp via ACT -> es bf16
        O'^T[65,q] += V'_h^T es  (V' carries a ones column -> row 64 = sums)
      V projection interleaved into pair0 cpass0 (V[j] before AV j).
      Per-pair normalization: sums rows -> DRAM -> reciprocal [4,512] ->
      bf16 -> DRAM -> partition-broadcast -> Onorm *= rec (pair 7 split
      into two [2,512] chains to shorten the tail).
  D)  out = Onorm^T @ w_proj + b  (lhsT=Onorm bf16, rhs=wp bf16)

softmax max-subtraction skipped: scores ~N(0, 0.33^2), safe for exp in fp32.
"""

import os
import sys
import types

sys.path.insert(0, "/opt/trn_rl_repo")

# The agent image's antenv lacks axon_hooks; register the NTFF profile hook
# shim so run_bass_kernel_spmd(trace=True) can capture exec_time_ns.
if "antenv.axon_hooks" not in sys.modules:
    _hooks = types.ModuleType("antenv.axon_hooks")
    _hook_store = [None]
    _hooks.set_axon_ntff_profile_hook = lambda h: _hook_store.__setitem__(0, h)
    _hooks.get_axon_ntff_profile_hook = lambda: _hook_store[0]
    sys.modules["antenv.axon_hooks"] = _hooks
    try:
        from trn_agent_boot.trn_boot import _ntff_profile_via_ctypes

        _hooks.set_axon_ntff_profile_hook(
            _ntff_profile_via_ctypes("/opt/axon/libaxon_pjrt.so")
        )
    except Exception:
        pass

import numpy as np
import concourse.bass as bass
import concourse.bacc as bacc
import concourse.mybir as mybir
import concourse.tile as tile
from concourse import masks
from concourse.bass_utils import run_bass_kernel_spmd

F32 = mybir.dt.float32
F32R = mybir.dt.float32r
BF16 = mybir.dt.bfloat16
EXP = mybir.ActivationFunctionType.Exp

B = 8
N = 1024  # sequence length
D = 1024  # embed dim
H = 16  # heads
HD = 64  # head dim
SCALE = HD**-0.5  # 0.125
NT = N // 128  # 8 seq tiles
DT = D // 128  # 8 embed tiles
NC2 = N // 512  # 2 free-dim chunks of 512
NPAIR = H // 2  # 8 head pairs

LAST_EXEC_NS = [None]


def build():
    nc = bacc.Bacc(None, target_bir_lowering=False)
    x = nc.declare_dram_parameter("x", [N, D], F32, isOutput=False)
    w_qkv = nc.declare_dram_parameter("w_qkv", [D, 3 * D], F32, isOutput=False)
    w_proj = nc.declare_dram_parameter("w_proj", [D, D], F32, isOutput=False)
    b_proj = nc.declare_dram_parameter("b_proj", [D], F32, isOutput=False)
    out = nc.declare_dram_parameter("out", [N, D], F32, isOutput=True)

    with tile.TileContext(nc) as tc:
        with (
            tc.tile_pool(name="const", bufs=1) as cpool,
            tc.tile_pool(name="xT", bufs=DT) as xTpool,
            tc.tile_pool(name="V", bufs=NT) as Vpool,
            tc.tile_pool(name="qkT", bufs=4) as qkTpool,
            tc.tile_pool(name="Onorm", bufs=DT) as Opool,
            tc.tile_pool(name="es", bufs=5) as espool,
            tc.tile_pool(name="wqk", bufs=4) as wqkpool,
            tc.tile_pool(name="wv", bufs=DT) as wvpool,
            tc.tile_pool(name="wp", bufs=DT) as wppool,
            tc.tile_pool(name="wst", bufs=4) as wstpool,
            tc.tile_pool(name="sums", bufs=7) as sumspool,
            tc.tile_pool(name="recb", bufs=3) as rbpool,
            tc.tile_pool(name="drs", bufs=1, space="DRAM") as drpool,
            tc.tile_pool(name="pq", bufs=1, space="PSUM") as pqpool,
        ):
            ident = cpool.tile([128, 128], F32, tag="ident")
            masks.make_identity(nc, ident[:])
            ident_r = cpool.tile([128, 128], F32R, tag="ident_r")
            nc.vector.tensor_copy(ident_r[:], ident[:])
            onescf = cpool.tile([128, H], F32, tag="onescf")
            nc.vector.memset(onescf[:], 1.0)
            b_bc = cpool.tile([128, D], F32, tag="b_bc")

            xT = [
                xTpool.tile([128, N], BF16, tag="xT", name=f"xT{j}")
                for j in range(DT)
            ]
            V = [
                Vpool.tile([128, H * (HD + 1)], BF16, tag="V", name=f"V{i}")
                for i in range(NT)
            ]
            Onorm = [
                Opool.tile([128, N], BF16, tag="On", name=f"On{i}")
                for i in range(DT)
            ]
            wv = [
                wvpool.tile([128, D], BF16, tag="wv", name=f"wv{j}")
                for j in range(DT)
            ]
            wp = [
                wppool.tile([128, D], BF16, tag="wp", name=f"wp{ct}")
                for ct in range(DT)
            ]
            sums_dram = drpool.tile([4 * NPAIR, 512], F32, tag="sumsd")
            rec_dram = drpool.tile([4 * NPAIR, 512], BF16, tag="recd")

            def stage_qk(pr):
                # enqueue fp32 stage DMAs for wq/wk column slices
                stgs = []
                for et in (pr, NPAIR + pr):
                    stg = wstpool.tile([128, D], F32, tag="wst", name=f"ws{et}")
                    src_w = w_qkv[:, et * 128 : (et + 1) * 128].rearrange(
                        "(j p) e -> p j e", p=128
                    )
                    nc.sync.dma_start(
                        stg[:].rearrange("p (j e) -> p j e", e=128), src_w
                    )
                    stgs.append(stg)
                return stgs

            def make_qk_emitter(pr, stgs=None):
                # cast staged weights to bf16 (DVE); emit(n) issues n matmuls
                # (32 total = 2 etiles x 2 chunks x 8 j)
                if stgs is None:
                    stgs = stage_qk(pr)
                wqs = []
                for k, et in enumerate((pr, NPAIR + pr)):
                    wq = wqkpool.tile([128, D], BF16, tag="wqk", name=f"wq{et}")
                    nc.vector.tensor_copy(wq[:], stgs[k][:])
                    wqs.append(wq)
                qts = [
                    qkTpool.tile([128, N], BF16, tag="qkT", name=f"qt{pr}"),
                    qkTpool.tile([128, N], BF16, tag="qkT", name=f"kt{pr}"),
                ]
                state = {"idx": 0, "pq": None}

                def emit(n):
                    for _ in range(n):
                        idx = state["idx"]
                        if idx >= 32:
                            return
                        g, j = idx // DT, idx % DT
                        ei, c = ((0, 0), (1, 0), (1, 1), (0, 1))[g]
                        if j == 0:
                            state["pq"] = pqpool.tile(
                                [128, 512], F32, tag="pq", name="pq"
                            )
                        nc.tensor.matmul(
                            state["pq"][:],
                            wqs[ei][:, j * 128 : (j + 1) * 128],
                            xT[j][:, c * 512 : (c + 1) * 512],
                            start=(j == 0),
                            stop=(j == DT - 1),
                        )
                        if j == DT - 1:
                            nc.vector.tensor_copy(
                                qts[ei][:, c * 512 : (c + 1) * 512],
                                state["pq"][:],
                            )
                        state["idx"] = idx + 1

                return emit, qts

            # ---- phase A: load x, cast bf16, transpose; stage weights ----
            with (
                tc.tile_pool(name="xin", bufs=6) as xpool,
                tc.tile_pool(name="tp", bufs=2, space="PSUM") as tppool,
            ):
                # DMA order: all of x first, then wq pair0, then wv, b_bc
                xts = []
                for i in range(4):
                    xt = xpool.tile([128, D], F32R, tag="x", name=f"x{i}")
                    nc.sync.dma_start(
                        xt[:], x[i * 128 : (i + 1) * 128, :].bitcast(F32R)
                    )
                    xts.append(xt)
                stg0 = stage_qk(0)
                for i in range(4, NT):
                    xt = xpool.tile([128, D], F32R, tag="x", name=f"x{i}")
                    nc.sync.dma_start(
                        xt[:], x[i * 128 : (i + 1) * 128, :].bitcast(F32R)
                    )
                    xts.append(xt)
                wvstg = []
                for j in range(DT):
                    stg = wstpool.tile([128, D], F32, tag="wst", name=f"wvs{j}")
                    nc.sync.dma_start(
                        stg[:], w_qkv[j * 128 : (j + 1) * 128, 2 * D : 3 * D]
                    )
                    wvstg.append(stg)
                nc.sync.dma_start(
                    b_bc[:].rearrange("p (a f) -> p a f", a=1),
                    b_proj[:].rearrange("(a n) -> a n", a=1).partition_broadcast(
                        128
                    ),
                )

                def transpose_tile(i):
                    for j in range(DT):
                        tp = tppool.tile([128, 128], F32R, tag="tp")
                        nc.tensor.transpose(
                            tp[:], xts[i][:, j * 128 : (j + 1) * 128], ident_r[:]
                        )
                        nc.vector.tensor_copy(
                            xT[j][:, i * 128 : (i + 1) * 128],
                            tp[:].bitcast(F32),
                        )

                for i in range(4):
                    transpose_tile(i)
                # wq0 casts land on DVE right after x0-3 copies
                emit0, qts0 = make_qk_emitter(0, stg0)
                for i in range(4, NT):
                    transpose_tile(i)

                # qk projection for pair 0: first q-c0 + k-c0 groups, so
                # scores can start; the rest drips into the step loop
                emit0(18)

                for j in range(DT):
                    nc.vector.tensor_copy(wv[j][:], wvstg[j][:])

                # ones columns of V' (col 64 of each 65-wide head group)
                for i in range(NT):
                    ones_view = V[i][:].rearrange("p (h e) -> p h e", e=HD + 1)
                    nc.vector.tensor_copy(
                        ones_view[:, :, HD : HD + 1],
                        onescf[:].rearrange("p (h e) -> p h e", e=1),
                    )

            # ---- phase C: attention steps, software pipelined ------------
            with (
                tc.tile_pool(name="s", bufs=2, space="PSUM") as spool,
                tc.tile_pool(name="o", bufs=3, space="PSUM") as opool,
            ):
                steps = [
                    (p, c, j)
                    for p in range(NPAIR)
                    for c in range(NC2)
                    for j in range(NT)
                ]
                qts_of = {0: qts0}
                emitters = {}

                def emit_score(s):
                    p, c, j = steps[s]
                    qt, kt = qts_of[p]
                    st = spool.tile([128, N], F32, tag="s", name="st")
                    cs = slice(c * 512, (c + 1) * 512)
                    for parity in range(2):
                        p0 = 64 * parity
                        nc.tensor.matmul(
                            st[:, 512 * parity : 512 * parity + 512],
                            kt[p0 : p0 + 64, j * 128 : (j + 1) * 128],
                            qt[p0 : p0 + 64, cs],
                            start=True,
                            stop=True,
                        )
                    return st

                def emit_vproj(i):
                    dst = V[i][:].rearrange("p (h e) -> p h e", e=HD + 1)
                    for cc in range(NC2):
                        pv = pqpool.tile([128, 512], F32, tag="pq", name="pv")
                        for jj in range(DT):
                            nc.tensor.matmul(
                                pv[:],
                                xT[jj][:, i * 128 : (i + 1) * 128],
                                wv[jj][:, cc * 512 : (cc + 1) * 512],
                                start=(jj == 0),
                                stop=(jj == DT - 1),
                            )
                        nc.vector.tensor_copy(
                            dst[:, 8 * cc : 8 * cc + 8, 0:HD],
                            pv[:].rearrange("p (h e) -> p h e", e=HD),
                        )

                def emit_chain(p, sld, base, nrows, eng):
                    rec = sumspool.tile([nrows, 512], F32, tag="rec", name="rec")
                    nc.vector.reciprocal(rec[:], sld[:])
                    r16 = sumspool.tile([nrows, 512], BF16, tag="r16", name="r16")
                    eng.tensor_copy(r16[:], rec[:])
                    nc.sync.dma_start(rec_dram[base : base + nrows, :], r16[:])
                    for r in range(nrows):
                        row = base + r
                        cc, parity = (row % 4) // 2, row % 2
                        p0 = 64 * parity
                        rb = rbpool.tile([128, 512], BF16, tag="rb")
                        half = rb[p0 : p0 + 64, :]
                        nc.sync.dma_start(
                            half.rearrange("p (a f) -> p a f", a=1),
                            rec_dram[row : row + 1, :].partition_broadcast(64),
                        )
                        osl = Onorm[p][p0 : p0 + 64, cc * 512 : (cc + 1) * 512]
                        eng.tensor_mul(osl, osl, half)

                pending = {}
                st_cur = emit_score(0)
                po = None
                sums_sb = None
                rec16 = None
                for s, (pair, c, j) in enumerate(steps):
                    if c == 0 and j == 0:
                        # pair start: prefetch next pair's wq; wp at pairs 2-5
                        if pair + 1 < NPAIR:
                            emitters[pair + 1] = make_qk_emitter(pair + 1)
                            qts_of[pair + 1] = emitters[pair + 1][1]
                        if 2 <= pair <= 5:
                            for ct in (2 * pair - 4, 2 * pair - 3):
                                stg = wstpool.tile(
                                    [128, D], F32, tag="wst", name=f"wps{ct}"
                                )
                                nc.sync.dma_start(
                                    stg[:],
                                    w_proj[ct * 128 : (ct + 1) * 128, :],
                                )
                                nc.gpsimd.tensor_copy(wp[ct][:], stg[:])
                    # exp of this step's scores (ACT paces the pipeline)
                    es = espool.tile([128, N], BF16, tag="es")
                    nc.scalar.activation(es[:], st_cur[:], EXP, scale=SCALE)

                    # lead: next step's score matmuls ahead of this AV
                    if s + 1 < len(steps):
                        st_cur = emit_score(s + 1)

                    if j == 0:
                        po = [
                            opool.tile([128, 512], F32, tag="o", name=f"po{k}")
                            for k in range(2)
                        ]
                    # V projection interleaved into pair0 cpass0
                    if pair == 0 and c == 0:
                        emit_vproj(j)

                    for parity in range(2):
                        h = 2 * pair + parity
                        nc.tensor.matmul(
                            po[parity][0 : HD + 1, :],
                            V[j][:, h * (HD + 1) : (h + 1) * (HD + 1)],
                            es[:, 512 * parity : 512 * parity + 512],
                            start=(j == 0),
                            stop=(j == NT - 1),
                        )
                    if pair == 0:
                        emit0(3)
                    if pair + 1 in emitters and (c * NT + j) >= 3:
                        emitters[pair + 1][0](3)

                    if j == NT - 1:
                        # drain po: O^T rows + sums row; per-cpass norm chain
                        cs = slice(c * 512, (c + 1) * 512)
                        for parity in range(2):
                            p0 = 64 * parity
                            nc.vector.tensor_copy(
                                Onorm[pair][p0 : p0 + 64, cs],
                                po[parity][0:HD, :],
                            )
                            srow = sumspool.tile([1, 512], F32, tag="srow")
                            nc.vector.tensor_copy(
                                srow[:], po[parity][HD : HD + 1, :]
                            )
                            row = 4 * pair + 2 * c + parity
                            nc.sync.dma_start(
                                sums_dram[row : row + 1, :], srow[:]
                            )
                        # enqueue the sums row-gather; the reciprocal +
                        # muls are deferred so they never block the pipe
                        if pair < NPAIR - 1:
                            if c == 1:
                                sld = sumspool.tile(
                                    [4, 512], F32, tag="sld", name="sld"
                                )
                                nc.sync.dma_start(
                                    sld[:], sums_dram[4 * pair : 4 * pair + 4, :]
                                )
                                pending[pair] = sld
                        else:
                            sld = sumspool.tile(
                                [2, 512], F32, tag="sld", name="sld7"
                            )
                            base = 4 * pair + 2 * c
                            nc.sync.dma_start(
                                sld[:], sums_dram[base : base + 2, :]
                            )
                            if c == 0:
                                pending[(NPAIR - 1, 0)] = sld
                            else:
                                # tail chain: all on DVE, minimal latency
                                emit_chain(pair, sld, base, 2, nc.vector)
                    if c == 0 and j == 2 and pair >= 1 and (pair - 1) in pending:
                        emit_chain(
                            pair - 1, pending.pop(pair - 1), 4 * (pair - 1),
                            4, nc.vector,
                        )
                    if (
                        c == 1
                        and j == 2
                        and pair == NPAIR - 1
                        and (NPAIR - 1, 0) in pending
                    ):
                        emit_chain(
                            pair, pending.pop((NPAIR - 1, 0)), 4 * pair, 2,
                            nc.vector,
                        )

            # ---- phase D: output projection + bias -----------------------
            with (
                tc.tile_pool(name="osb", bufs=2) as osbpool,
                tc.tile_pool(name="dp", bufs=4, space="PSUM") as dppool,
            ):
                for i in range(NT):
                    pd = [
                        dppool.tile([128, 512], F32, tag="dp", name=f"dp{ec}")
                        for ec in range(NC2)
                    ]
                    for ct in range(DT):
                        for ec in range(NC2):
                            nc.tensor.matmul(
                                pd[ec][:],
                                Onorm[ct][:, i * 128 : (i + 1) * 128],
                                wp[ct][:, ec * 512 : (ec + 1) * 512],
                                start=(ct == 0),
                                stop=(ct == DT - 1),
                            )
                    ot = osbpool.tile([128, D], F32, tag="osb")
                    for ec in range(NC2):
                        nc.vector.tensor_add(
                            ot[:, ec * 512 : (ec + 1) * 512],
                            pd[ec][:],
                            b_bc[:, ec * 512 : (ec + 1) * 512],
                        )
                    nc.sync.dma_start(out[i * 128 : (i + 1) * 128, :], ot[:])

    nc.compile()
    return nc


_NC = [None]


def _get_nc():
    if _NC[0] is None:
        _NC[0] = build()
    return _NC[0]


def kernel(x, w_qkv, w_proj, b_proj):
    x = np.asarray(x, dtype=np.float32)
    w_qkv = np.asarray(w_qkv, dtype=np.float32)
    w_proj = np.asarray(w_proj, dtype=np.float32)
    b_proj = np.asarray(b_proj, dtype=np.float32)
    assert x.shape == (B, N, D)

    nc = _get_nc()
    in_maps = [
        {"x": x[i], "w_qkv": w_qkv, "w_proj": w_proj, "b_proj": b_proj}
        for i in range(B)
    ]
    trace = os.environ.get("KERNEL_TRACE") == "1"
    res = run_bass_kernel_spmd(
        nc, in_maps, core_ids=list(range(B)), trace=trace
    )
    LAST_EXEC_NS[0] = res.exec_time_ns
    return np.stack([res.results[i]["out"] for i in range(B)], axis=0)
